# revision 11
# baseline (speedup 1.0000x reference)
"""Trainium2 Bass kernel for seq2seq LSTM encoder/decoder cross-entropy loss.

Strategy (8 NeuronCores, SPMD):
  - LSTM encoder (50 steps) + decoder (49 steps) replicated on all cores.
  - Output projection (512 -> 32000 vocab) tensor-parallel: each core owns a
    4000-row vocab shard of W_out/b_out and computes per-(step,batch) partial
    sum-of-exp(logits) plus the target-token logit for targets in its shard.
  - Host combines: loss = sum_t mean_b( ln(sum_c s_c) - sum_c tau_c ).

Perf design (v2b):
  - fp8e4 DoubleRow matmuls (2 K-tiles per instruction) halve PE stream
    cycles and LDWEIGHTS count vs bf16. Weights pre-scaled (x16 / x8) into
    fp8 range, compensated via the activation `scale` input. DoubleRow
    requires destination partition 0, so gates stay in [64, 2048] layout.
  - Sigmoid eliminated: sig(x) = (1+tanh(x/2))/2 folded into the cell
    algebra with scalar_tensor_tensor fused ops, carrying C=2c and H=2h.
    The g-gate weight columns carry an extra 2x so ALL gates use one
    tanh(psum/32) scale. All scalar funcs are {tanh, exp} -> one ACT table,
    no reloads even when projection exp interleaves with the cell steps.
  - x_t added on the PE via identity-matmul accumulation into gates PSUM.
  - H-transposes write into the tail of the gates PSUM tile (o-gate bank,
    dead after its tanh read) -> no extra PSUM bank; frees 4 banks for the
    interleaved projection round.
  - Interleaving: decoder X=x@Wih tiles are computed during encoder steps;
    one projection round (12 DR matmuls + one [128,2048] exp with accum)
    runs inside each decoder step's dependency gaps. Keeps the PE p-state
    high (2.4 GHz needs ~3us of continuous execution) and hides P1/P4.
"""

import functools
import numpy as np
import ml_dtypes

import concourse.bacc as bacc
import concourse.bass as bass
import concourse.mybir as mybir
from concourse import tile

BF16 = mybir.dt.bfloat16
F32 = mybir.dt.float32
FP8 = mybir.dt.float8e4
I32 = mybir.dt.int32
DR = mybir.MatmulPerfMode.DoubleRow

H = 512
KC = 4  # hidden chunks of 128
B = 64
G = 4 * H  # 2048 gates
VOCAB = 32000
NCORES = 8
AF = mybir.ActivationFunctionType

W_SCALE = 16.0  # fp8 weight pre-scale on the x side (W_ih); W_hh/W_out use 8


def _cfg(enc_steps=50, dec_steps=49, v_ntiles=8):
    assert enc_steps % 2 == 0
    mt = (dec_steps * B + 127) // 128
    return dict(
        enc_steps=enc_steps,
        dec_steps=dec_steps,
        v_ntiles=v_ntiles,
        vs_pad=v_ntiles * 512,
        emb_rows=VOCAB,
        enc_mt=enc_steps * B // 128,
        dec_mt=mt,
        exp_rounds=2,  # P4 rounds (one [128,2048] exp each) per M-tile
    )


def build_program(cfg):
    enc_steps, dec_steps = cfg["enc_steps"], cfg["dec_steps"]
    enc_mt, dec_mt = cfg["enc_mt"], cfg["dec_mt"]
    vnt, vsp = cfg["v_ntiles"], cfg["vs_pad"]
    emb_rows = cfg["emb_rows"]
    n_exp = cfg["exp_rounds"]

    nc = bacc.Bacc(
        "TRN2", target_bir_lowering=False, debug=False, num_devices=NCORES
    )

    def din(name, shape, dt):
        return nc.dram_tensor(name, list(shape), dt, kind="ExternalInput").ap()

    def dout(name, shape, dt):
        return nc.dram_tensor(name, list(shape), dt, kind="ExternalOutput").ap()

    emb_in = din("emb_in", (emb_rows, H), BF16)
    emb_tgt = din("emb_tgt", (emb_rows, H), BF16)
    # W^T in f,i,g,o whole-gate column order; g columns x2
    wih_enc = din("wih_enc", (H, G), FP8)
    whh_enc = din("whh_enc", (H, G), FP8)
    wih_dec = din("wih_dec", (H, G), FP8)
    whh_dec = din("whh_dec", (H, G), FP8)
    bias_enc = din("bias_enc", (128, G), F32)
    bias_dec = din("bias_dec", (128, G), F32)
    wout_t = din("wout_t", (H, vsp), FP8)  # W_out^T shard x8, padded
    brow_pair = din("brow_pair", (1, 2 * vsp), FP8)  # [b_out x16 | zeros]
    waug = din("waug", (vsp, 516), F32)  # [W_sh | b_sh | 0 0 0] unscaled
    ident = din("ident", (128, 128), BF16)
    ident32 = din("ident32", (B, B), F32)
    idmm = din("idmm", (128, B), BF16)  # [I64; I64] stacked
    ones_pair = din("ones_pair", (1, 256), FP8)  # [ones(128) | zeros(128)]
    etok = din("etok", (128, enc_mt), I32)
    dtok = din("dtok", (128, dec_mt), I32)
    ttok = din("ttok", (B, dec_steps), I32)

    s_out = dout("s_out", (128, dec_mt * n_exp), F32)
    t_out = dout("t_out", (B, dec_steps), F32)

    xih_dec = nc.dram_tensor("xih_dec", [dec_mt * 128, G], BF16, kind="Internal").ap()

    with tile.TileContext(nc) as tc:
        with tc.tile_pool(name="persist", bufs=1) as pp, \
             tc.tile_pool(name="ps_main", bufs=1, space="PSUM") as pmain:
            whh_e = pp.tile([128, KC * G], FP8)
            whh_d = pp.tile([128, KC * G], FP8)
            wout_s = pp.tile([128, KC * vsp], FP8)
            id_sb = pp.tile([128, 128], BF16)
            id32_sb = pp.tile([B, B], F32)
            idmm_sb = pp.tile([128, B], BF16)
            ones_sb = pp.tile([1, 256], FP8)
            brow_sb = pp.tile([1, 2 * vsp], FP8)
            etok_sb = pp.tile([128, enc_mt], I32)
            dtok_sb = pp.tile([128, dec_mt], I32)
            ttok_sb = pp.tile([B, dec_steps], I32)
            hT_all = pp.tile([128, KC * dec_mt * 128], FP8)
            s_acc = pp.tile([128, dec_mt * n_exp], F32)
            t_acc = pp.tile([B, dec_steps], F32)

            for k in range(KC):
                nc.sync.dma_start(whh_e[:, k * G:(k + 1) * G], whh_enc[k * 128:(k + 1) * 128, :])
                nc.sync.dma_start(whh_d[:, k * G:(k + 1) * G], whh_dec[k * 128:(k + 1) * 128, :])
                nc.sync.dma_start(wout_s[:, k * vsp:(k + 1) * vsp], wout_t[k * 128:(k + 1) * 128, :])
            nc.sync.dma_start(id_sb[:], ident[:])
            nc.sync.dma_start(id32_sb[:], ident32[:])
            nc.sync.dma_start(idmm_sb[:], idmm[:])
            nc.sync.dma_start(ones_sb[:], ones_pair[:])
            nc.sync.dma_start(brow_sb[:], brow_pair[:])
            nc.sync.dma_start(etok_sb[:], etok[:])
            nc.sync.dma_start(dtok_sb[:], dtok[:])
            nc.sync.dma_start(ttok_sb[:], ttok[:])

            whh_e_v = whh_e[:].rearrange("p (k g) -> p k g", k=KC)
            whh_d_v = whh_d[:].rearrange("p (k g) -> p k g", k=KC)
            wout_v = wout_s[:].rearrange("p (k v) -> p k v", k=KC)
            hT_all_v = hT_all[:].rearrange("p (k t) -> p k t", k=KC)
            ones_v = ones_sb[:].rearrange("p (j m) -> p j m", j=2)
            brow_v = brow_sb[:].rearrange("p (j v) -> p j v", j=2)

            # gates PSUM [128, 2048] f32 (4 banks). Cell math uses rows 0:64;
            # H-transposes land in cols 1792:2048 (o-gate region, dead after
            # its tanh read) across all 128 partitions.
            gp = pmain.tile([128, G], F32)
            TPS0 = 1792  # transpose scratch base col inside gp

            with tc.tile_pool(name="mn", bufs=1) as mn:
                state = {"hT_v": None, "c": None}
                hT0 = mn.tile([128, KC * B], FP8, tag="hT0")
                nc.gpsimd.memset(hT0[:], 0.0)
                c0 = mn.tile([B, H], F32, tag="c0")
                nc.gpsimd.memset(c0[:], 0.0)
                state["hT_v"] = hT0[:].rearrange("p (k b) -> p k b", k=KC)
                state["c"] = c0

                def lstm_step(t, is_dec, xg_tile):
                    """One cell step. xg_tile: enc SBUF ring tile [128, G]
                    (half = t%2) or dec xt tile [64, G] (rows 0:64)."""
                    whh_v = whh_d_v if is_dec else whh_e_v
                    half = 0 if is_dec else (t % 2) * B

                    # gates[64, 2048] = 16*(h@Whh + x@Wih + b)  [x32 for g]
                    for p in (0, 2):
                        for blk in range(4):
                            nc.tensor.matmul(
                                out=gp[0:B, blk * 512:(blk + 1) * 512],
                                lhsT=state["hT_v"][:, p:p + 2, :],
                                rhs=whh_v[:, p:p + 2, blk * 512:(blk + 1) * 512],
                                start=(p == 0), stop=False,
                                perf_mode=DR,
                            )
                    for blk in range(4):
                        nc.tensor.matmul(
                            out=gp[0:B, blk * 512:(blk + 1) * 512],
                            lhsT=idmm_sb[half:half + B, :],
                            rhs=xg_tile[half:half + B, blk * 512:(blk + 1) * 512],
                            start=False, stop=True,
                        )

                    # col order f,i,g,o: one tanh over f,i,g then o
                    tfig = mn.tile([B, 3 * H], BF16, tag="tfig", bufs=2)
                    nc.scalar.activation(out=tfig[:], in_=gp[0:B, 0:3 * H],
                                         func=AF.Tanh, scale=1.0 / 32.0)
                    to = mn.tile([B, H], BF16, tag="to", bufs=2)
                    nc.scalar.activation(out=to[:], in_=gp[0:B, 3 * H:4 * H],
                                         func=AF.Tanh, scale=1.0 / 32.0)

                    # C = 0.5*(tf+1)*C_prev + (ti+1)*tg
                    pP = mn.tile([B, H], F32, tag="pP", bufs=2)
                    nc.vector.scalar_tensor_tensor(
                        out=pP[:], in0=tfig[:, 0:H], scalar=1.0,
                        in1=state["c"][:],
                        op0=mybir.AluOpType.add, op1=mybir.AluOpType.mult)
                    qQ = mn.tile([B, H], F32, tag="qQ", bufs=2)
                    nc.vector.scalar_tensor_tensor(
                        out=qQ[:], in0=tfig[:, H:2 * H], scalar=1.0,
                        in1=tfig[:, 2 * H:3 * H],
                        op0=mybir.AluOpType.add, op1=mybir.AluOpType.mult)
                    c_new = mn.tile([B, H], F32, tag="c", bufs=2)
                    nc.vector.scalar_tensor_tensor(
                        out=c_new[:], in0=pP[:], scalar=0.5, in1=qQ[:],
                        op0=mybir.AluOpType.mult, op1=mybir.AluOpType.add)
                    tc_ = mn.tile([B, H], BF16, tag="tc", bufs=2)
                    nc.scalar.activation(out=tc_[:], in_=c_new[:],
                                         func=AF.Tanh, scale=0.5)
                    # H = 2h = (to+1)*tanh(c), f32 so the transpose can
                    # reuse the f32 gates PSUM tile as scratch
                    hh = mn.tile([B, H], F32, tag="h", bufs=3)
                    nc.vector.scalar_tensor_tensor(
                        out=hh[:], in0=to[:], scalar=1.0, in1=tc_[:],
                        op0=mybir.AluOpType.add, op1=mybir.AluOpType.mult)

                    # transpose H -> hT chunks [128, 64] into gp tail
                    for k in range(KC):
                        nc.tensor.transpose(
                            out=gp[:, TPS0 + k * B:TPS0 + (k + 1) * B],
                            in_=hh[:, k * 128:(k + 1) * 128],
                            identity=id32_sb[:],
                        )
                    if is_dec:
                        hT_dst = hT_all_v[:, :, t * B:(t + 1) * B]
                        nc.vector.tensor_copy(out=hT_dst, in_=gp[:, TPS0:TPS0 + KC * B])
                        state["hT_v"] = hT_all_v[:, :, t * B:(t + 1) * B]
                        # tau: gather W_out rows of target tokens, fused dot
                        wt = mn.tile([B, 516], F32, tag="wt", bufs=3)
                        nc.gpsimd.indirect_dma_start(
                            out=wt[:], out_offset=None, in_=waug[:],
                            in_offset=bass.IndirectOffsetOnAxis(
                                ap=ttok_sb[:, t:t + 1], axis=0),
                        )
                        prod = mn.tile([B, H], F32, tag="prod", bufs=2)
                        nc.vector.scalar_tensor_tensor(
                            out=prod[:], in0=hh[:], scalar=0.5, in1=wt[:, 0:H],
                            op0=mybir.AluOpType.mult, op1=mybir.AluOpType.mult)
                        tau0 = mn.tile([B, 1], F32, tag="tau0", bufs=2)
                        nc.vector.tensor_reduce(
                            out=tau0[:], in_=prod[:],
                            axis=mybir.AxisListType.X, op=mybir.AluOpType.add)
                        nc.vector.tensor_add(
                            out=t_acc[:, t:t + 1], in0=tau0[:],
                            in1=wt[:, 512:513])
                    else:
                        hT_new = mn.tile([128, KC * B], FP8, tag="hTs", bufs=2)
                        nc.vector.tensor_copy(out=hT_new[:], in_=gp[:, TPS0:TPS0 + KC * B])
                        state["hT_v"] = hT_new[:].rearrange("p (k b) -> p k b", k=KC)
                    state["c"] = c_new

                # ---------------- encoder phase (+ P1 interleave) ----------
                with tc.tile_pool(name="p1", bufs=1) as p1, \
                     tc.tile_pool(name="p1_ps", bufs=1, space="PSUM") as p1p:
                    wih_e = p1.tile([128, KC * G], FP8)
                    wih_d = p1.tile([128, KC * G], FP8)
                    bias_e = p1.tile([128, G], F32)
                    bias_d = p1.tile([128, G], F32)
                    for k in range(KC):
                        nc.sync.dma_start(wih_e[:, k * G:(k + 1) * G], wih_enc[k * 128:(k + 1) * 128, :])
                        nc.sync.dma_start(wih_d[:, k * G:(k + 1) * G], wih_dec[k * 128:(k + 1) * 128, :])
                    nc.sync.dma_start(bias_e[:], bias_enc[:])
                    nc.sync.dma_start(bias_d[:], bias_dec[:])
                    wih_e_v = wih_e[:].rearrange("p (k g) -> p k g", k=KC)
                    wih_d_v = wih_d[:].rearrange("p (k g) -> p k g", k=KC)

                    def emit_p1_tile(src, mt):
                        """Gather+transpose+matmul one X tile [128, G].
                        Returns the SBUF xg tile (enc) or None (dec, DMA'd)."""
                        is_dec = src == 1
                        xr = p1.tile([128, H], BF16, tag="xr", bufs=3)
                        nc.gpsimd.indirect_dma_start(
                            out=xr[:], out_offset=None,
                            in_=(emb_tgt if is_dec else emb_in)[:],
                            in_offset=bass.IndirectOffsetOnAxis(
                                ap=(dtok_sb if is_dec else etok_sb)[:, mt:mt + 1],
                                axis=0),
                        )
                        tpx = p1p.tile([128, H], BF16, tag="tpx", bufs=1)
                        for k in range(KC):
                            nc.tensor.transpose(
                                out=tpx[:, k * 128:(k + 1) * 128],
                                in_=xr[:, k * 128:(k + 1) * 128],
                                identity=id_sb[:],
                            )
                        xT8 = p1.tile([128, H], FP8, tag="xT8", bufs=3)
                        nc.vector.tensor_copy(out=xT8[:], in_=tpx[:])
                        xT8_v = xT8[:].rearrange("p (k c) -> p k c", k=KC)
                        wv = wih_d_v if is_dec else wih_e_v
                        bias = bias_d if is_dec else bias_e
                        xg = p1.tile([128, G], BF16,
                                     tag=("xgd" if is_dec else "xge"), bufs=4)
                        for r in range(2):
                            xp = p1p.tile([128, 1024], F32, tag="xp", bufs=1)
                            for p in (0, 2):
                                for n in range(2):
                                    c0_ = r * 1024 + n * 512
                                    nc.tensor.matmul(
                                        out=xp[:, n * 512:(n + 1) * 512],
                                        lhsT=xT8_v[:, p:p + 2, :],
                                        rhs=wv[:, p:p + 2, c0_:c0_ + 512],
                                        start=(p == 0), stop=(p == 2),
                                        perf_mode=DR,
                                    )
                            nc.vector.tensor_add(
                                out=xg[:, r * 1024:(r + 1) * 1024], in0=xp[:],
                                in1=bias[:, r * 1024:(r + 1) * 1024])
                        if is_dec:
                            nc.sync.dma_start(
                                xih_dec[mt * 128:(mt + 1) * 128, :], xg[:])
                            return None
                        return xg

                    enc_ring = {}
                    for m in range(min(3, enc_mt)):
                        enc_ring[m] = emit_p1_tile(0, m)
                    for t in range(enc_steps):
                        lstm_step(t, False, enc_ring[t // 2])
                        if t % 2 == 0:
                            m = t // 2 + 3
                            if m < enc_mt:
                                enc_ring[m] = emit_p1_tile(0, m)
                        else:
                            m = (t - 1) // 2
                            if m < dec_mt:
                                emit_p1_tile(1, m)

                # ---------------- decoder phase (+ P4 interleave) ----------
                with tc.tile_pool(name="p4_ps", bufs=1, space="PSUM") as p4p:
                    if dec_steps * B < dec_mt * 128:
                        nc.gpsimd.memset(
                            hT_all_v[:, :, dec_steps * B:dec_mt * 128], 0.0)

                    def emit_p4_round(mt, r):
                        lp = p4p.tile([128, 2048], F32, tag="lp", bufs=1)
                        hT_mt = [hT_all_v[:, p:p + 2, mt * 128:(mt + 1) * 128]
                                 for p in (0, 2)]
                        for pi, p in enumerate((0, 2)):
                            for n in range(4):
                                v0 = (r * 4 + n) * 512
                                nc.tensor.matmul(
                                    out=lp[:, n * 512:(n + 1) * 512],
                                    lhsT=hT_mt[pi],
                                    rhs=wout_v[:, p:p + 2, v0:v0 + 512],
                                    start=(p == 0), stop=False,
                                    perf_mode=DR,
                                )
                        for n in range(4):
                            v0 = (r * 4 + n) * 512
                            nc.tensor.matmul(
                                out=lp[:, n * 512:(n + 1) * 512],
                                lhsT=ones_v[:, :, :],
                                rhs=brow_v[:, :, v0:v0 + 512],
                                start=False, stop=True,
                                perf_mode=DR,
                            )
                        ex = mn.tile([128, 2048], BF16, tag="ex", bufs=2)
                        nc.scalar.activation(
                            out=ex[:], in_=lp[:], func=AF.Exp, scale=1.0 / 16.0,
                            accum_out=s_acc[:, mt * n_exp + r:mt * n_exp + r + 1],
                        )

                    for u in range(dec_steps):
                        xt = mn.tile([B, G], BF16, tag="xt", bufs=3)
                        nc.sync.dma_start(xt[:], xih_dec[u * B:(u + 1) * B, :])
                        lstm_step(u, True, xt)
                        if u >= 2:
                            emit_p4_round((u - 2) // 2, (u - 2) % 2)
                    for mt, r in [((dec_steps - 1) // 2, 1), (dec_mt - 1, 0),
                                  (dec_mt - 1, 1)]:
                        emit_p4_round(mt, r)

            nc.sync.dma_start(s_out[:], s_acc[:])
            nc.sync.dma_start(t_out[:], t_acc[:])

    nc.compile()
    return nc


# ============================ host side ============================

# original gate order in W_*: i(0:512), f(512:1024), g(1024:1536), o(1536:2048)
_f, _i, _g, _o = np.r_[512:1024], np.r_[0:512], np.r_[1024:1536], np.r_[1536:2048]
_PERM2 = np.r_[_f, _i, _g, _o]  # whole-gate order f,i,g,o
_GBOOST = np.ones(G, np.float32)
_GBOOST[1024:1536] = 2.0  # g gate: x2 so tanh(psum/32) = tanh(g)
_FP8 = mybir.dt.np(FP8)


def _prep_shared(inputs, cfg):
    bf = ml_dtypes.bfloat16
    enc_steps, dec_steps = cfg["enc_steps"], cfg["dec_steps"]
    enc_mt, dec_mt = cfg["enc_mt"], cfg["dec_mt"]

    def wT8(w, scale):
        wt = np.asarray(w, np.float32)[_PERM2].T * scale  # [H, G]
        return np.ascontiguousarray(wt * _GBOOST[None, :]).astype(_FP8)

    def biasb(bi, bh):
        b = (np.asarray(bi, np.float32) + np.asarray(bh, np.float32))[_PERM2]
        b = b * W_SCALE * _GBOOST
        return np.ascontiguousarray(np.broadcast_to(b, (128, G))).astype(np.float32)

    il = np.asarray(inputs["input_lines"]).astype(np.int64)[:enc_steps]
    tl = np.asarray(inputs["target_lines"]).astype(np.int64)[: dec_steps + 1]
    etok_flat = il.reshape(-1)
    dtok_flat = tl[:-1].reshape(-1)
    dtok_flat = np.concatenate([
        dtok_flat, np.zeros(dec_mt * 128 - dtok_flat.size, np.int64)])
    tgt_next = tl[1:].reshape(-1)

    idmm = np.zeros((128, B), np.float32)
    idmm[0:B, :] = np.eye(B)
    idmm[B:128, :] = np.eye(B)
    ones_pair = np.zeros((1, 256), np.float32)
    ones_pair[0, 0:128] = 1.0

    shared = {
        "emb_in": np.asarray(inputs["emb_in"], np.float32).astype(bf),
        "emb_tgt": np.asarray(inputs["emb_tgt"], np.float32).astype(bf),
        "wih_enc": wT8(inputs["W_ih_enc"], W_SCALE),
        "whh_enc": wT8(inputs["W_hh_enc"], W_SCALE / 2),
        "wih_dec": wT8(inputs["W_ih_dec"], W_SCALE),
        "whh_dec": wT8(inputs["W_hh_dec"], W_SCALE / 2),
        "bias_enc": biasb(inputs["b_ih_enc"], inputs["b_hh_enc"]),
        "bias_dec": biasb(inputs["b_ih_dec"], inputs["b_hh_dec"]),
        "ident": np.eye(128, dtype=bf),
        "ident32": np.eye(B, dtype=np.float32),
        "idmm": idmm.astype(bf),
        "ones_pair": ones_pair.astype(_FP8),
        "etok": np.ascontiguousarray(
            etok_flat.reshape(enc_mt, 128).T).astype(np.int32),
        "dtok": np.ascontiguousarray(
            dtok_flat.reshape(dec_mt, 128).T).astype(np.int32),
    }
    return shared, tgt_next


def _prep_core(inputs, cfg, core, tgt_next):
    vnt, vsp = cfg["v_ntiles"], cfg["vs_pad"]
    dec_steps = cfg["dec_steps"]
    W_out = np.asarray(inputs["W_out"], np.float32)
    b_out = np.asarray(inputs["b_out"], np.float32)
    vocab = W_out.shape[0]
    vs = vocab // NCORES
    lo, hi = core * vs, (core + 1) * vs
    w_sh = W_out[lo:hi]
    b_sh = b_out[lo:hi]

    wout_t = np.zeros((H, vsp), np.float32)
    wout_t[:, :vs] = w_sh.T * (W_SCALE / 2)
    brow_pair = np.zeros((1, 2 * vsp), np.float32)
    brow_pair[0, :vsp] = -448.0  # pad cols: exp(-28) ~ 0
    brow_pair[0, :vs] = b_sh * W_SCALE

    waug = np.zeros((vsp, 516), np.float32)
    waug[:vs, :H] = w_sh
    waug[:vs, 512] = b_sh

    tloc = tgt_next - lo
    tloc = np.where((tloc >= 0) & (tloc < vs), tloc, vs).astype(np.int32)
    ttok = np.ascontiguousarray(tloc.reshape(dec_steps, B).T).astype(np.int32)

    return {
        "wout_t": wout_t.astype(_FP8),
        "brow_pair": brow_pair.astype(_FP8),
        "waug": waug,
        "ttok": ttok,
    }


def make_in_maps(inputs, cfg):
    shared, tgt_next = _prep_shared(inputs, cfg)
    return [dict(shared, **_prep_core(inputs, cfg, c, tgt_next))
            for c in range(NCORES)]


def combine(results, cfg):
    dec_steps, dec_mt = cfg["dec_steps"], cfg["dec_mt"]
    n_exp = cfg["exp_rounds"]
    S = np.zeros((128, dec_mt * n_exp), np.float64)
    T = np.zeros((B, dec_steps), np.float64)
    for r in results:
        S += np.asarray(r["s_out"], np.float64)
        T += np.asarray(r["t_out"], np.float64)
    S = S.reshape(128, dec_mt, n_exp).sum(axis=2)
    s_tb = np.transpose(S.reshape(2, 64, dec_mt), (2, 0, 1)).reshape(-1, 64)
    s_tb = s_tb[:dec_steps]
    t_tb = T.T  # [dec_steps, B]
    loss = np.sum(np.mean(np.log(s_tb) - t_tb, axis=1))
    return np.float32(loss)


@functools.lru_cache(maxsize=2)
def _get_compiled(key):
    cfg = _cfg()
    nc = build_program(cfg)
    return cfg, nc


def _run_hw(nc, in_maps):
    from concourse.bass_utils import run_bass_kernel_spmd
    res = run_bass_kernel_spmd(nc, in_maps, core_ids=list(range(NCORES)),
                               trace=False)
    return res.results


def kernel(**inputs):
    cfg, nc = _get_compiled("full")
    in_maps = make_in_maps(inputs, cfg)
    results = _run_hw(nc, in_maps)
    return combine(results, cfg)
